# revision 61
# baseline (speedup 1.0000x reference)
"""Trainium2 kernel for nn_DirectForce (gnn_message_passing).

Math (see reference):
    h   = softplus(X @ w1 + b1) - log(2)          per-edge MLP        [E, 64]
    mag = h @ w2 + b2                                                  [E, 1]
    mag = mag - mean_over_center(mag)[center]      scatter-mean debias
    pair-average mag between each directed edge and its reverse edge
    F   = segment_sum(mag * unit_vec, center)                          [N, 3]

The pair keys (center+neigh+length+|unit|) are identical exactly for the two
directions of each undirected edge (reverse edge has negated vector, same
length), so the sorted-pair averaging pairs each edge with its reverse.  Since
unit_rev = -unit, the pair-averaged scatter reduces algebraically to
    F = segsum(0.5*mag*unit, center) - segsum(0.5*mag*unit, neigh)
which removes the argsort entirely (verified to 2.5e-8 vs the reference).

Device (8 NeuronCores, SPMD, edges partitioned contiguously 200k/core):
  - features pre-transposed, cast to fp16, and pre-tiled on host to
    [NTILES, 128, XT_TILE] per core so every input DMA tile is contiguous in
    HBM; fp16 halves the dominant HBM traffic (819MB -> 410MB total,
    ~51MB/core ~= 143us at the ~358GB/s per-core HBM limit).
  - mm1 in fp16: two matmuls per superchunk with zero-padded weights [w1;0]
    and [0;w1] accumulate stacked z = [zA; zB] [128, 512] f32 in PSUM.
    fp16 weights use the separate-LDWEIGHTS path, which the PE overlaps with
    in-flight matmuls via the background weight buffer (f32r could not:
    self-loading matmul serializes LDW+MM, which was the old 367us wall).
  - softplus: two ACT passes (this act_info has no Softplus table): Exp
    (bias=b1) per [128, 1024] z pair (two adjacent PSUM banks) into an fp16
    e tile, then Ln(x+1) in [128, 4096] halves of a 16-superchunk group.
    Wide ops amortize the ACT per-op overhead; both functions pinned to the
    one table set holding both (else the compiler reloads tables, 50x1.5us).
    ACT is the bottleneck (~189us busy): exp/ln have no single-table
    replacement, no other engine evaluates transcendentals, and a
    relu+Gaussian single-pass approximation fails the 2e-2 gate (6.3e-2).
  - mm2: per h 512-slice, one matmul with a [128, 32] fp16 block-diag w2
    variant (nonzero cols 2v:2v+2) accumulating into a shared [32, 512] f32
    PSUM bank across 16 superchunks; rows it doesn't own get +0.  mm2s are
    TRICKLED <=2 per pair, >=2 pairs after their Ln half: an 8-deep mm2
    burst at the group boundary blocks later mm1s in the PE's strict FIFO,
    starves ACT, and idles the PE enough that HAM re-throttles it to
    1.2GHz.  The bank takes ONE batched DVE copy + DMA per 16k edges (DMA
    has no PSUM port on TRN2); its row layout makes host decode a single
    flat reshape.
  - startup: dummy activation hoists the ACT table load; b1s rides the
    scalar HWDGE queue (not behind weights on gpsimd); first input tile
    covers a full superchunk pair.
Host post (index-structured tail, ~6% of input bytes, numpy):
  - debias via bincount, unit vectors, the two segment sums above.
"""

import numpy as np

N_ATOMS = 50000
E_TOT = 1600000
D_FEAT = 128
H_DIM = 64
N_CORES = 8
EC = E_TOT // N_CORES          # 200000 edges per core
SC = 1024                      # edges per superchunk (2 chunks of 512)
NSC = (EC + SC - 1) // SC      # 196 superchunks
ECP = NSC * SC                 # 200704 padded edges per core
XT_TILE = 8192                 # edges per input DMA tile
GRP = 16                       # superchunks per mag PSUM bank (32 rows)
NMG = (NSC + GRP - 1) // GRP   # 13 mag banks (last one quarter-filled)

# input-tile taper: small tiles at the head (compute starts fast; the first
# tile covers a full superchunk *pair* so one DMA unblocks the first Exp)
# and tail (short pipeline drain), big 8-superchunk tiles between
TILE_SIZES = [2, 2, 4] + [8] * 22 + [4, 2, 2, 2, 1, 1]
assert sum(TILE_SIZES) == NSC
NTILES = len(TILE_SIZES)

_CACHE = {}
LAST_RESULTS = None


def _patch_act_tables():
    """Make Exp and Ln resolve to the single table set that contains both
    (natural_log_exp_and_others) so the ACT table is loaded exactly once;
    the default per-op greedy choice flip-flops between exp_and_friends and
    natural_log, paying ~1.5us per reload.  Table-set ids are positional,
    so keys/order are preserved."""
    import functools
    import concourse.hw_specs as hw_specs
    import concourse.bacc as bacc_mod
    import concourse.mybir as mybir

    if _CACHE.get("tables_patched"):
        return
    orig = hw_specs.get_activation_tables
    Exp = mybir.ActivationFunctionType.Exp
    Ln = mybir.ActivationFunctionType.Ln

    def patched(arch):
        out = {}
        for name, fns in orig(arch).items():
            if name != "natural_log_exp_and_others":
                fns = fns - {Exp, Ln}
            out[name] = fns
        return out

    cached = functools.cache(patched)
    hw_specs.get_activation_tables = cached
    bacc_mod.get_activation_tables = cached
    _CACHE["tables_patched"] = True


def _build_nc():
    import concourse.bacc as bacc
    import concourse.mybir as mybir
    import concourse.tile as tile

    _patch_act_tables()

    F32 = mybir.dt.float32
    F16 = mybir.dt.float16
    Exp = mybir.ActivationFunctionType.Exp
    Ln = mybir.ActivationFunctionType.Ln

    nc = bacc.Bacc("TRN2", target_bir_lowering=False, debug=False)
    xt_d = nc.dram_tensor("xt", [NTILES, 128, XT_TILE], F16, kind="ExternalInput")
    w1a_d = nc.dram_tensor("w1a", [128, 128], F16, kind="ExternalInput")
    w1b_d = nc.dram_tensor("w1b", [128, 128], F16, kind="ExternalInput")
    b1_d = nc.dram_tensor("b1s", [128, 1], F32, kind="ExternalInput")
    w2v_d = nc.dram_tensor("w2v", [128, GRP * 32], F16, kind="ExternalInput")
    mag_d = nc.dram_tensor("mag", [NMG, 32, 512], F32, kind="ExternalOutput")

    with tile.TileContext(nc) as tc:
        with (
            tc.tile_pool(name="wp", bufs=1) as wp,
            tc.tile_pool(name="xp", bufs=4) as xp,
            tc.tile_pool(name="ep", bufs=2) as ep,
            tc.tile_pool(name="hp", bufs=2) as hp,
            tc.tile_pool(name="mp", bufs=3) as mp,
            tc.tile_pool(name="zp", bufs=3, space="PSUM") as zp,
            tc.tile_pool(name="magp", bufs=2, space="PSUM") as magp,
        ):
            # dummy early activation: hoists the ~1.5us ACT table load off
            # the first real activation's dependency chain
            dummy = wp.tile([128, 1], F32, tag="dummy")
            nc.vector.memset(dummy[:], 0.0)
            nc.scalar.activation(dummy[:], dummy[:], Exp, bias=0.0)

            w1a = wp.tile([128, 128], F16, tag="w1a")
            w1b = wp.tile([128, 128], F16, tag="w1b")
            b1s = wp.tile([128, 1], F32, tag="b1s")
            w2v = wp.tile([128, GRP * 32], F16, tag="w2v")
            nc.gpsimd.dma_start(w1a[:], w1a_d[:])
            nc.gpsimd.dma_start(w1b[:], w1b_d[:])
            # b1s on the scalar HWDGE queue: it gates the first Exp and must
            # not wait behind the weight transfers on the gpsimd SWDGE queue
            nc.scalar.dma_start(b1s[:], b1_d[:])
            nc.gpsimd.dma_start(w2v[:], w2v_d[:])

            z_ps = None
            e_sb = None
            h_state = [None]
            MM2_DELAY = 4      # pairs between Ln-half and mm2 eligibility
            pending_mm2 = []   # [(pair#, h tile, v, stop_bank, grp idx)]

            mag_state = [None]

            def trickle_mm2(pair_no, limit):
                # emit up to `limit` single mm2 matmuls whose Ln-half was
                # issued >= MM2_DELAY pairs ago.  Spreading mm2s keeps the
                # PE instruction stream uniform (no 8-deep mm2 bursts that
                # block later mm1s in the strict FIFO and starve ACT) and
                # avoids the HAM re-throttle from bursty PE idling.
                n = 0
                while pending_mm2 and n < limit and (
                    pair_no is None
                    or pair_no - pending_mm2[0][0] >= MM2_DELAY
                ):
                    _, h_sb, v, stop_bank, gi = pending_mm2.pop(0)
                    if v == 0:
                        mag_new = magp.tile([32, 512], F32, tag="mag")
                        mag_state[0] = mag_new
                    mag_ps = mag_state[0]
                    nc.tensor.matmul(
                        mag_ps[:], w2v[:, 32 * v:32 * (v + 1)],
                        h_sb[:, v * 512:(v + 1) * 512],
                        start=(v == 0), stop=stop_bank,
                        skip_group_check=True,
                    )
                    if stop_bank:
                        # bank complete: DMA has no PSUM port on TRN2 ->
                        # one batched DVE copy per bank, then DMA from SBUF
                        mag_sb = mp.tile([32, 512], F32, tag="magsb")
                        nc.vector.tensor_copy(mag_sb[:], mag_ps[:])
                        nc.gpsimd.dma_start(mag_d[gi], mag_sb[:])
                    n += 1

            g = 0
            for ti, size in enumerate(TILE_SIZES):
                width = size * SC
                xt = xp.tile([128, XT_TILE], F16, tag="xt")
                nc.sync.dma_start(xt[:, :width], xt_d[ti, :, :width])
                for t in range(size):
                    off = t * SC
                    j = g % GRP          # superchunk slot within the group
                    ei = g % 2           # slot within the z pair tile
                    if ei == 0:
                        z_ps = zp.tile([128, 1024], F32, tag="z")
                    zs = z_ps[:, ei * 512:(ei + 1) * 512]
                    nc.tensor.matmul(
                        zs, w1a[:], xt[:, off:off + 512],
                        start=True, stop=False,
                    )
                    nc.tensor.matmul(
                        zs, w1b[:], xt[:, off + 512:off + 1024],
                        start=False, stop=True,
                    )
                    if ei == 1:
                        trickle_mm2(g // 2, 2)
                        q = (j - 1) // 2     # pair slot within the group
                        if q == 0:
                            e_sb = ep.tile([128, GRP * 512], F16, tag="e")
                        nc.scalar.activation(
                            e_sb[:, q * 1024:(q + 1) * 1024], z_ps[:],
                            Exp, bias=b1s[:, :1],
                        )
                        half = GRP // 4   # pairs per Ln half-op
                        if (q + 1) % half == 0 or g + 1 == NSC:
                            q0 = (q // half) * half   # first pair this half
                            if q0 == 0:
                                h_new = hp.tile([128, GRP * 512], F16, tag="h")
                                h_state[0] = h_new
                            hcur = h_state[0]
                            nc.scalar.activation(
                                hcur[:, q0 * 1024:(q + 1) * 1024],
                                e_sb[:, q0 * 1024:(q + 1) * 1024],
                                Ln, bias=1.0,
                            )
                            bank_done = (j == GRP - 1) or (g + 1 == NSC)
                            v1 = 2 * (q + 1)
                            for v in range(2 * q0, v1):
                                pending_mm2.append(
                                    (g // 2, hcur, v,
                                     bank_done and v == v1 - 1, g // GRP)
                                )
                    g += 1
            trickle_mm2(None, len(pending_mm2))
    nc.compile()
    return nc


def _get_nc():
    if "nc" not in _CACHE:
        _CACHE["nc"] = _build_nc()
    return _CACHE["nc"]


def kernel(features, edge_vectors, edge_lengths, edge_index, w1, b1, w2, b2):
    global LAST_RESULTS
    from concourse.bass_utils import run_bass_kernel_spmd

    features = np.asarray(features, dtype=np.float32)
    edge_vectors = np.asarray(edge_vectors, dtype=np.float32)
    edge_lengths = np.asarray(edge_lengths, dtype=np.float32)
    edge_index = np.asarray(edge_index)
    w1 = np.asarray(w1, dtype=np.float32)
    b1 = np.asarray(b1, dtype=np.float32).reshape(-1)
    w2 = np.asarray(w2, dtype=np.float32).reshape(-1, 1)
    b2 = np.asarray(b2, dtype=np.float32).reshape(-1)

    # replicated small weights, padded for the stacked-z / block-diag tricks
    w1a = np.zeros((128, 128), np.float16)
    w1a[:, :H_DIM] = w1
    w1b = np.zeros((128, 128), np.float16)
    w1b[:, H_DIM:] = w1
    b1s = np.concatenate([b1, b1]).astype(np.float32).reshape(128, 1)
    # mm2 variant v (cols 32v:32v+32) owns mag-bank rows 2v:2v+2: within the
    # variant, col 2v+r contracts w2 against the r-th 64-partition half of h
    w2v = np.zeros((128, GRP * 32), np.float16)
    for v in range(GRP):
        w2v[:H_DIM, 32 * v + 2 * v] = w2[:, 0]
        w2v[H_DIM:, 32 * v + 2 * v + 1] = w2[:, 0]

    # shard edges contiguously across cores; per-core transposed fp16 panel
    in_maps = []
    for c in range(N_CORES):
        sl = slice(c * EC, (c + 1) * EC)
        panel = np.zeros((128, ECP), np.float16)
        panel[:, :EC] = features[sl].T
        xt = np.zeros((NTILES, 128, XT_TILE), np.float16)
        a = 0
        for ti, size in enumerate(TILE_SIZES):
            w = size * SC
            xt[ti, :, :w] = panel[:, a:a + w]
            a += w
        in_maps.append({"xt": xt, "w1a": w1a, "w1b": w1b, "b1s": b1s, "w2v": w2v})

    nc = _get_nc()
    try:
        res = run_bass_kernel_spmd(nc, in_maps, core_ids=list(range(N_CORES)))
    except Exception:
        # one retry for transient runtime failures
        import time
        time.sleep(2.0)
        res = run_bass_kernel_spmd(nc, in_maps, core_ids=list(range(N_CORES)))
    LAST_RESULTS = res

    # decode mag: out [NMG, 32, 512]; flat index (G*16 + j)*1024 + r*512 + c
    # equals the edge index directly, so decode is a flat reshape
    mag = np.empty(E_TOT, np.float32)
    for c in range(N_CORES):
        arr = res.results[c]["mag"]  # [NMG, 32, 512]
        mag[c * EC:(c + 1) * EC] = arr.reshape(-1)[:EC]

    # fold b2 and the shifted-softplus constant: h_ref = h_dev - log(2)
    mag = mag + (b2[0] - np.float32(np.log(2.0)) * w2.sum())

    center = edge_index[0].astype(np.int64)
    neigh = edge_index[1].astype(np.int64)

    # scatter-mean debias per center atom
    cnt = np.bincount(center, minlength=N_ATOMS).astype(np.float32)
    ssum = np.bincount(center, weights=mag.astype(np.float64), minlength=N_ATOMS)
    bias = (ssum / np.maximum(cnt, 1.0)).astype(np.float32)
    mag = mag - bias[center]

    # pair-averaged antisymmetric force assembly (see module docstring)
    unit = edge_vectors / edge_lengths[:, None]
    val = (0.5 * mag)[:, None] * unit  # [E, 3]
    forces = np.zeros((N_ATOMS, 3), np.float32)
    for k in range(3):
        fc = np.bincount(center, weights=val[:, k].astype(np.float64), minlength=N_ATOMS)
        fn = np.bincount(neigh, weights=val[:, k].astype(np.float64), minlength=N_ATOMS)
        forces[:, k] = (fc - fn).astype(np.float32)
    return forces


# revision 62
# speedup vs baseline: 1.0025x; 1.0025x over previous
"""Trainium2 kernel for nn_DirectForce (gnn_message_passing).

Math (see reference):
    h   = softplus(X @ w1 + b1) - log(2)          per-edge MLP        [E, 64]
    mag = h @ w2 + b2                                                  [E, 1]
    mag = mag - mean_over_center(mag)[center]      scatter-mean debias
    pair-average mag between each directed edge and its reverse edge
    F   = segment_sum(mag * unit_vec, center)                          [N, 3]

The pair keys (center+neigh+length+|unit|) are identical exactly for the two
directions of each undirected edge (reverse edge has negated vector, same
length), so the sorted-pair averaging pairs each edge with its reverse.  Since
unit_rev = -unit, the pair-averaged scatter reduces algebraically to
    F = segsum(0.5*mag*unit, center) - segsum(0.5*mag*unit, neigh)
which removes the argsort entirely (verified to 2.5e-8 vs the reference).

Device (8 NeuronCores, SPMD, edges partitioned contiguously 200k/core):
  - features pre-transposed, cast to fp16, and pre-tiled on host to
    [NTILES, 128, XT_TILE] per core so every input DMA tile is contiguous in
    HBM; fp16 halves the dominant HBM traffic (819MB -> 410MB total,
    ~51MB/core ~= 143us at the ~358GB/s per-core HBM limit).
  - mm1 in fp16: two matmuls per superchunk with zero-padded weights [w1;0]
    and [0;w1] accumulate stacked z = [zA; zB] [128, 512] f32 in PSUM.
    fp16 weights use the separate-LDWEIGHTS path, which the PE overlaps with
    in-flight matmuls via the background weight buffer (f32r could not:
    self-loading matmul serializes LDW+MM, which was the old 367us wall).
  - softplus: two ACT passes (this act_info has no Softplus table): Exp
    (bias=b1) per [128, 1024] z pair (two adjacent PSUM banks) into an fp16
    e tile, then Ln(x+1) in [128, 4096] halves of a 16-superchunk group.
    Wide ops amortize the ACT per-op overhead; both functions pinned to the
    one table set holding both (else the compiler reloads tables, 50x1.5us).
    ACT is the bottleneck (~189us busy): exp/ln have no single-table
    replacement, no other engine evaluates transcendentals, and a
    relu+Gaussian single-pass approximation fails the 2e-2 gate (6.3e-2).
  - mm2: per h 512-slice, one matmul with a [128, 32] fp16 block-diag w2
    variant (nonzero cols 2v:2v+2) accumulating into a shared [32, 512] f32
    PSUM bank across 16 superchunks; rows it doesn't own get +0.  mm2s are
    TRICKLED <=2 per pair, >=2 pairs after their Ln half: an 8-deep mm2
    burst at the group boundary blocks later mm1s in the PE's strict FIFO,
    starves ACT, and idles the PE enough that HAM re-throttles it to
    1.2GHz.  The bank takes ONE batched DVE copy + DMA per 16k edges (DMA
    has no PSUM port on TRN2); its row layout makes host decode a single
    flat reshape.
  - startup: dummy activation hoists the ACT table load; b1s rides the
    scalar HWDGE queue (not behind weights on gpsimd); first input tile
    covers a full superchunk pair.
Host post (index-structured tail, ~6% of input bytes, numpy):
  - debias via bincount, unit vectors, the two segment sums above.
"""

import numpy as np

N_ATOMS = 50000
E_TOT = 1600000
D_FEAT = 128
H_DIM = 64
N_CORES = 8
EC = E_TOT // N_CORES          # 200000 edges per core
SC = 1024                      # edges per superchunk (2 chunks of 512)
NSC = (EC + SC - 1) // SC      # 196 superchunks
ECP = NSC * SC                 # 200704 padded edges per core
XT_TILE = 8192                 # edges per input DMA tile
GRP = 16                       # superchunks per mag PSUM bank (32 rows)
NMG = (NSC + GRP - 1) // GRP   # 13 mag banks (last one quarter-filled)

# input-tile taper: small tiles at the head (compute starts fast; the first
# tile covers a full superchunk *pair* so one DMA unblocks the first Exp)
# and tail (short pipeline drain), big 8-superchunk tiles between
TILE_SIZES = [2, 2, 4] + [8] * 22 + [4, 2, 2, 2, 1, 1]
assert sum(TILE_SIZES) == NSC
NTILES = len(TILE_SIZES)

_CACHE = {}
LAST_RESULTS = None


def _patch_act_tables():
    """Make Exp and Ln resolve to the single table set that contains both
    (natural_log_exp_and_others) so the ACT table is loaded exactly once;
    the default per-op greedy choice flip-flops between exp_and_friends and
    natural_log, paying ~1.5us per reload.  Table-set ids are positional,
    so keys/order are preserved."""
    import functools
    import concourse.hw_specs as hw_specs
    import concourse.bacc as bacc_mod
    import concourse.mybir as mybir

    if _CACHE.get("tables_patched"):
        return
    orig = hw_specs.get_activation_tables
    Exp = mybir.ActivationFunctionType.Exp
    Ln = mybir.ActivationFunctionType.Ln

    def patched(arch):
        out = {}
        for name, fns in orig(arch).items():
            if name != "natural_log_exp_and_others":
                fns = fns - {Exp, Ln}
            out[name] = fns
        return out

    cached = functools.cache(patched)
    hw_specs.get_activation_tables = cached
    bacc_mod.get_activation_tables = cached
    _CACHE["tables_patched"] = True


def _build_nc():
    import concourse.bacc as bacc
    import concourse.mybir as mybir
    import concourse.tile as tile

    _patch_act_tables()

    F32 = mybir.dt.float32
    F16 = mybir.dt.float16
    Exp = mybir.ActivationFunctionType.Exp
    Ln = mybir.ActivationFunctionType.Ln

    nc = bacc.Bacc("TRN2", target_bir_lowering=False, debug=False)
    xt_d = nc.dram_tensor("xt", [NTILES, 128, XT_TILE], F16, kind="ExternalInput")
    w1a_d = nc.dram_tensor("w1a", [128, 128], F16, kind="ExternalInput")
    w1b_d = nc.dram_tensor("w1b", [128, 128], F16, kind="ExternalInput")
    b1_d = nc.dram_tensor("b1s", [128, 1], F32, kind="ExternalInput")
    w2v_d = nc.dram_tensor("w2v", [128, GRP * 32], F16, kind="ExternalInput")
    mag_d = nc.dram_tensor("mag", [NMG, 32, 512], F32, kind="ExternalOutput")

    with tile.TileContext(nc) as tc:
        with (
            tc.tile_pool(name="wp", bufs=1) as wp,
            tc.tile_pool(name="xp", bufs=4) as xp,
            tc.tile_pool(name="ep", bufs=2) as ep,
            tc.tile_pool(name="hp", bufs=2) as hp,
            tc.tile_pool(name="mp", bufs=3) as mp,
            tc.tile_pool(name="zp", bufs=3, space="PSUM") as zp,
            tc.tile_pool(name="magp", bufs=2, space="PSUM") as magp,
        ):
            # dummy early activation: hoists the ~1.5us ACT table load off
            # the first real activation's dependency chain
            dummy = wp.tile([128, 1], F32, tag="dummy")
            nc.vector.memset(dummy[:], 0.0)
            nc.scalar.activation(dummy[:], dummy[:], Exp, bias=0.0)

            w1a = wp.tile([128, 128], F16, tag="w1a")
            w1b = wp.tile([128, 128], F16, tag="w1b")
            b1s = wp.tile([128, 1], F32, tag="b1s")
            w2v = wp.tile([128, GRP * 32], F16, tag="w2v")
            nc.gpsimd.dma_start(w1a[:], w1a_d[:])
            nc.gpsimd.dma_start(w1b[:], w1b_d[:])
            # b1s on the scalar HWDGE queue: it gates the first Exp and must
            # not wait behind the weight transfers on the gpsimd SWDGE queue
            nc.scalar.dma_start(b1s[:], b1_d[:])
            nc.gpsimd.dma_start(w2v[:], w2v_d[:])

            z_ps = None
            e_sb = None
            h_state = [None]
            MM2_DELAY = 3      # pairs between Ln-half and mm2 eligibility
            pending_mm2 = []   # [(pair#, h tile, v, stop_bank, grp idx)]

            mag_state = [None]

            def trickle_mm2(pair_no, limit):
                # emit up to `limit` single mm2 matmuls whose Ln-half was
                # issued >= MM2_DELAY pairs ago.  Spreading mm2s keeps the
                # PE instruction stream uniform (no 8-deep mm2 bursts that
                # block later mm1s in the strict FIFO and starve ACT) and
                # avoids the HAM re-throttle from bursty PE idling.
                n = 0
                while pending_mm2 and n < limit and (
                    pair_no is None
                    or pair_no - pending_mm2[0][0] >= MM2_DELAY
                ):
                    _, h_sb, v, stop_bank, gi = pending_mm2.pop(0)
                    if v == 0:
                        mag_new = magp.tile([32, 512], F32, tag="mag")
                        mag_state[0] = mag_new
                    mag_ps = mag_state[0]
                    nc.tensor.matmul(
                        mag_ps[:], w2v[:, 32 * v:32 * (v + 1)],
                        h_sb[:, v * 512:(v + 1) * 512],
                        start=(v == 0), stop=stop_bank,
                        skip_group_check=True,
                    )
                    if stop_bank:
                        # bank complete: DMA has no PSUM port on TRN2 ->
                        # one batched DVE copy per bank, then DMA from SBUF
                        mag_sb = mp.tile([32, 512], F32, tag="magsb")
                        nc.vector.tensor_copy(mag_sb[:], mag_ps[:])
                        nc.gpsimd.dma_start(mag_d[gi], mag_sb[:])
                    n += 1

            g = 0
            for ti, size in enumerate(TILE_SIZES):
                width = size * SC
                xt = xp.tile([128, XT_TILE], F16, tag="xt")
                nc.sync.dma_start(xt[:, :width], xt_d[ti, :, :width])
                for t in range(size):
                    off = t * SC
                    j = g % GRP          # superchunk slot within the group
                    ei = g % 2           # slot within the z pair tile
                    if ei == 0:
                        z_ps = zp.tile([128, 1024], F32, tag="z")
                    zs = z_ps[:, ei * 512:(ei + 1) * 512]
                    nc.tensor.matmul(
                        zs, w1a[:], xt[:, off:off + 512],
                        start=True, stop=False,
                    )
                    nc.tensor.matmul(
                        zs, w1b[:], xt[:, off + 512:off + 1024],
                        start=False, stop=True,
                    )
                    if ei == 1:
                        trickle_mm2(g // 2, 2)
                        q = (j - 1) // 2     # pair slot within the group
                        if q == 0:
                            e_sb = ep.tile([128, GRP * 512], F16, tag="e")
                        nc.scalar.activation(
                            e_sb[:, q * 1024:(q + 1) * 1024], z_ps[:],
                            Exp, bias=b1s[:, :1],
                        )
                        half = GRP // 4   # pairs per Ln half-op
                        if (q + 1) % half == 0 or g + 1 == NSC:
                            q0 = (q // half) * half   # first pair this half
                            if q0 == 0:
                                h_new = hp.tile([128, GRP * 512], F16, tag="h")
                                h_state[0] = h_new
                            hcur = h_state[0]
                            nc.scalar.activation(
                                hcur[:, q0 * 1024:(q + 1) * 1024],
                                e_sb[:, q0 * 1024:(q + 1) * 1024],
                                Ln, bias=1.0,
                            )
                            bank_done = (j == GRP - 1) or (g + 1 == NSC)
                            v1 = 2 * (q + 1)
                            for v in range(2 * q0, v1):
                                pending_mm2.append(
                                    (g // 2, hcur, v,
                                     bank_done and v == v1 - 1, g // GRP)
                                )
                    g += 1
            trickle_mm2(None, len(pending_mm2))
    nc.compile()
    return nc


def _get_nc():
    if "nc" not in _CACHE:
        _CACHE["nc"] = _build_nc()
    return _CACHE["nc"]


def kernel(features, edge_vectors, edge_lengths, edge_index, w1, b1, w2, b2):
    global LAST_RESULTS
    from concourse.bass_utils import run_bass_kernel_spmd

    features = np.asarray(features, dtype=np.float32)
    edge_vectors = np.asarray(edge_vectors, dtype=np.float32)
    edge_lengths = np.asarray(edge_lengths, dtype=np.float32)
    edge_index = np.asarray(edge_index)
    w1 = np.asarray(w1, dtype=np.float32)
    b1 = np.asarray(b1, dtype=np.float32).reshape(-1)
    w2 = np.asarray(w2, dtype=np.float32).reshape(-1, 1)
    b2 = np.asarray(b2, dtype=np.float32).reshape(-1)

    # replicated small weights, padded for the stacked-z / block-diag tricks
    w1a = np.zeros((128, 128), np.float16)
    w1a[:, :H_DIM] = w1
    w1b = np.zeros((128, 128), np.float16)
    w1b[:, H_DIM:] = w1
    b1s = np.concatenate([b1, b1]).astype(np.float32).reshape(128, 1)
    # mm2 variant v (cols 32v:32v+32) owns mag-bank rows 2v:2v+2: within the
    # variant, col 2v+r contracts w2 against the r-th 64-partition half of h
    w2v = np.zeros((128, GRP * 32), np.float16)
    for v in range(GRP):
        w2v[:H_DIM, 32 * v + 2 * v] = w2[:, 0]
        w2v[H_DIM:, 32 * v + 2 * v + 1] = w2[:, 0]

    # shard edges contiguously across cores; per-core transposed fp16 panel
    in_maps = []
    for c in range(N_CORES):
        sl = slice(c * EC, (c + 1) * EC)
        panel = np.zeros((128, ECP), np.float16)
        panel[:, :EC] = features[sl].T
        xt = np.zeros((NTILES, 128, XT_TILE), np.float16)
        a = 0
        for ti, size in enumerate(TILE_SIZES):
            w = size * SC
            xt[ti, :, :w] = panel[:, a:a + w]
            a += w
        in_maps.append({"xt": xt, "w1a": w1a, "w1b": w1b, "b1s": b1s, "w2v": w2v})

    nc = _get_nc()
    try:
        res = run_bass_kernel_spmd(nc, in_maps, core_ids=list(range(N_CORES)))
    except Exception:
        # one retry for transient runtime failures
        import time
        time.sleep(2.0)
        res = run_bass_kernel_spmd(nc, in_maps, core_ids=list(range(N_CORES)))
    LAST_RESULTS = res

    # decode mag: out [NMG, 32, 512]; flat index (G*16 + j)*1024 + r*512 + c
    # equals the edge index directly, so decode is a flat reshape
    mag = np.empty(E_TOT, np.float32)
    for c in range(N_CORES):
        arr = res.results[c]["mag"]  # [NMG, 32, 512]
        mag[c * EC:(c + 1) * EC] = arr.reshape(-1)[:EC]

    # fold b2 and the shifted-softplus constant: h_ref = h_dev - log(2)
    mag = mag + (b2[0] - np.float32(np.log(2.0)) * w2.sum())

    center = edge_index[0].astype(np.int64)
    neigh = edge_index[1].astype(np.int64)

    # scatter-mean debias per center atom
    cnt = np.bincount(center, minlength=N_ATOMS).astype(np.float32)
    ssum = np.bincount(center, weights=mag.astype(np.float64), minlength=N_ATOMS)
    bias = (ssum / np.maximum(cnt, 1.0)).astype(np.float32)
    mag = mag - bias[center]

    # pair-averaged antisymmetric force assembly (see module docstring)
    unit = edge_vectors / edge_lengths[:, None]
    val = (0.5 * mag)[:, None] * unit  # [E, 3]
    forces = np.zeros((N_ATOMS, 3), np.float32)
    for k in range(3):
        fc = np.bincount(center, weights=val[:, k].astype(np.float64), minlength=N_ATOMS)
        fn = np.bincount(neigh, weights=val[:, k].astype(np.float64), minlength=N_ATOMS)
        forces[:, k] = (fc - fn).astype(np.float32)
    return forces
